# revision 4
# baseline (speedup 1.0000x reference)
"""Trainium2 Bass kernel for nn_ComplexGraphAttentionLayer.

Strategy: token-parallel over 8 cores (cores 0-3 -> batch 0, 4-7 -> batch 1;
each core owns 256 tokens). K/V are projected locally then AllGathered within
each 4-core batch group. The q/k projections and the QK^T score matmuls use
true fp32 (scores get squared then softmaxed -- FP22 is not enough there);
everything else uses float32r (FP22 reads at bf16 speed).
"""

import numpy as np
import ml_dtypes

import concourse.bass as bass
import concourse.bacc as bacc
import concourse.mybir as mybir
import concourse.tile as tile
from concourse.bass_utils import run_bass_kernel_spmd
from concourse.masks import make_identity

P = 128
B, S, D_MODEL, H, HD = 2, 1024, 1024, 16, 64
F2 = 2 * D_MODEL          # 2048 feature dim (real|imag)
N_CORES = 8
GROUP = 4                 # cores per batch
TOK = S // GROUP          # 256 tokens per core
TT = TOK // P             # 2 token tiles
FCH = F2 // P             # 16 feature chunks
KC = S // P               # 8 key chunks
LN_EPS = 1e-5
ALPHA = 8.0 ** -0.25      # folded 1/sqrt(HD)^(1/2) per operand -> scores /8

f32 = mybir.dt.float32
f32r = mybir.dt.float32r
bf16 = mybir.dt.bfloat16

_cache = {}
_last_result = None


def _head_perm():
    """Column order: for h: [yr_h (64), yi_h (64)] -> interleaved by head."""
    idx = []
    for h in range(H):
        idx.extend(range(h * HD, (h + 1) * HD))
        idx.extend(range(D_MODEL + h * HD, D_MODEL + (h + 1) * HD))
    return np.array(idx)


def _effT(p, scale=1.0):
    """W_eff.T for y = [xr|xi] @ W_eff.T, W_eff = [[Wr,-Wi],[Wi,Wr]]."""
    wr = np.asarray(p["Wr"], np.float32) * scale
    wi = np.asarray(p["Wi"], np.float32) * scale
    return np.ascontiguousarray(
        np.block([[wr.T, wi.T], [-wi.T, wr.T]]).astype(np.float32)
    )


def _bias_eff(p, scale=1.0):
    return np.concatenate(
        [np.asarray(p["br"], np.float32), np.asarray(p["bi"], np.float32)]
    ).astype(np.float32) * scale


def _build_program(flags):
    nc = bacc.Bacc("TRN2", target_bir_lowering=False, debug=False,
                   num_devices=N_CORES)

    io = {}
    io["xT"] = nc.dram_tensor("xT", [F2, TOK], f32, kind="ExternalInput")
    io["xnat"] = nc.dram_tensor("xnat", [TOK, F2], f32, kind="ExternalInput")
    io["biasc"] = nc.dram_tensor("biasc", [H, TOK, S], bf16, kind="ExternalInput")
    io["wqT"] = nc.dram_tensor("wqT", [F2, F2], f32, kind="ExternalInput")
    io["wkT"] = nc.dram_tensor("wkT", [F2, F2], f32, kind="ExternalInput")
    io["wvT"] = nc.dram_tensor("wvT", [F2, F2], f32, kind="ExternalInput")
    io["woT"] = nc.dram_tensor("woT", [F2, F2], f32, kind="ExternalInput")
    io["w1T"] = nc.dram_tensor("w1T", [F2, 4 * F2], f32, kind="ExternalInput")
    io["w2T"] = nc.dram_tensor("w2T", [4 * F2, F2], f32, kind="ExternalInput")
    if flags["any_bias"]:
        io["bvec"] = nc.dram_tensor("bvec", [5, F2], f32, kind="ExternalInput")
        io["b1vec"] = nc.dram_tensor("b1vec", [1, 4 * F2], f32,
                                     kind="ExternalInput")
    if flags["ln_affine"]:
        io["lnv"] = nc.dram_tensor("lnv", [4, F2], f32, kind="ExternalInput")
    io["yout"] = nc.dram_tensor("yout", [TOK, F2], f32, kind="ExternalOutput")

    with tile.TileContext(nc) as tc:
        _emit(tc, nc, io, flags)
    nc.compile()
    return nc


def _emit(tc, nc, io, flags):
    from contextlib import ExitStack

    es = ExitStack()
    with es:
        singles = es.enter_context(tc.tile_pool(name="singles", bufs=1))
        sheets = es.enter_context(tc.tile_pool(name="sheets", bufs=5))
        work = es.enter_context(tc.tile_pool(name="work", bufs=2))
        wpool = es.enter_context(tc.tile_pool(name="wpool", bufs=3))
        cbp = es.enter_context(tc.tile_pool(name="cbp", bufs=3))
        small = es.enter_context(tc.tile_pool(name="small", bufs=4))
        dram = es.enter_context(tc.tile_pool(name="dram", bufs=1, space="DRAM"))
        psum = es.enter_context(tc.tile_pool(name="psum", bufs=4, space="PSUM"))

        def ps512(name):
            return psum.tile([P, 512], f32, tag="mm512", space="PSUM",
                             bufs=4, name=name)

        def ps128(name):
            return psum.tile([P, P], f32, tag="t128", space="PSUM", bufs=2,
                             name=name)

        ident = singles.tile([P, P], f32)
        make_identity(nc, ident)
        eps_t = singles.tile([P, 1], f32)
        nc.vector.memset(eps_t, LN_EPS)

        lnv_sb = None
        bvec_sb = None
        b1vec_sb = None
        if flags["ln_affine"]:
            lnv_sb = singles.tile([1, 4, F2], f32)
            nc.sync.dma_start(out=lnv_sb, in_=io["lnv"][None, :, :])
        if flags["any_bias"]:
            bvec_sb = singles.tile([1, 5, F2], f32)
            nc.sync.dma_start(out=bvec_sb, in_=io["bvec"][None, :, :])
            b1vec_sb = singles.tile([1, 4 * F2], f32)
            nc.sync.dma_start(out=b1vec_sb, in_=io["b1vec"][0, :][None, :])

        def bcast(row_ap):
            """[1, n] SBUF row -> [P, n] partition-broadcast AP."""
            a = row_ap
            return bass.AP(tensor=a.tensor, offset=a.offset,
                           ap=[[0, P]] + [list(x) for x in a.ap[1:]])

        kv_in = dram.tile([TOK, 2 * F2], f32)
        kv_full = dram.tile([S, 2 * F2], f32)

        xT = sheets.tile([P, FCH, TOK], f32, tag="sheet", name="xT_sb")
        nc.sync.dma_start(out=xT, in_=io["xT"].rearrange("(f p) t -> p f t", p=P))
        xnat = sheets.tile([P, TT, F2], f32, tag="sheet", name="xnat_sb")
        nc.sync.dma_start(out=xnat,
                          in_=io["xnat"].rearrange("(t p) f -> p t f", p=P))
        qT = sheets.tile([P, H, TOK], f32, tag="sheet", name="qT_sb")

        def proj(wT_dram, out_cols, fch, lhsT_fn, consumer, use_f32r,
                 bias_row=None):
            """out[tok, out_cols] += lhsT.T @ wT, streamed in 1024-col chunks.

            Weights stream f-outer (one [P,2,512] tile live at a time);
            4 PSUM groups (2 chunk-halves x 2 token tiles) accumulate in
            parallel across the f loop.
            """
            for dq in range(out_cols // 1024):
                pss = [[ps512(f"pp_{dq}_{d2}_{t}") for t in range(TT)]
                       for d2 in range(2)]
                for f in range(fch):
                    wdt = f32r if use_f32r else f32
                    w = wpool.tile([P, 2, 512], wdt, tag="w", name="w_sb")
                    src_ap = wT_dram[f * P:(f + 1) * P,
                                     dq * 1024:(dq + 1) * 1024]
                    if use_f32r:
                        src_ap = src_ap.bitcast(f32r)
                    nc.sync.dma_start(
                        out=w, in_=src_ap.rearrange("p (d n) -> p d n", d=2))
                    for d2 in range(2):
                        for t in range(TT):
                            nc.tensor.matmul(pss[d2][t], lhsT_fn(f, t),
                                             w[:, d2, :],
                                             start=(f == 0),
                                             stop=(f == fch - 1))
                for d2 in range(2):
                    for t in range(TT):
                        d512 = dq * 2 + d2
                        if bias_row is not None:
                            nc.vector.tensor_tensor(
                                pss[d2][t], pss[d2][t],
                                bcast(bias_row[:, d512 * 512:(d512 + 1) * 512]),
                                op=mybir.AluOpType.add)
                        consumer(pss[d2][t], t, d512)

        def xT_lhsT(f, t):
            return xT[:, f, t * P:(t + 1) * P]

        bias_q = bias_k = bias_v = bias_o = bias_f1 = bias_f2 = None
        if flags["any_bias"]:
            bias_q = bvec_sb[:, 0, :]
            bias_k = bvec_sb[:, 1, :]
            bias_v = bvec_sb[:, 2, :]
            bias_o = bvec_sb[:, 3, :]
            bias_f2 = bvec_sb[:, 4, :]
            bias_f1 = b1vec_sb

        # ---------------- Phase A: k, v projections + AllGather ----------
        ktmp = sheets.tile([P, TT, F2], f32, tag="sheet", name="ktmp_sb")

        def k_consumer(ps, t, d):
            nc.vector.tensor_copy(ktmp[:, t, d * 512:(d + 1) * 512], ps)

        proj(io["wkT"], F2, FCH, xT_lhsT, k_consumer, False, bias_row=bias_k)
        nc.sync.dma_start(
            out=kv_in[:, 0:F2].rearrange("(t p) f -> p t f", p=P), in_=ktmp)

        vtmp = sheets.tile([P, TT, F2], f32, tag="sheet", name="vtmp_sb")

        def v_consumer(ps, t, d):
            nc.vector.tensor_copy(vtmp[:, t, d * 512:(d + 1) * 512], ps)

        proj(io["wvT"], F2, FCH, xT_lhsT, v_consumer, False, bias_row=bias_v)
        nc.sync.dma_start(
            out=kv_in[:, F2:2 * F2].rearrange("(t p) f -> p t f", p=P),
            in_=vtmp)

        nc.gpsimd.collective_compute(
            "AllGather",
            mybir.AluOpType.bypass,
            replica_groups=[[0, 1, 2, 3], [4, 5, 6, 7]],
            ins=[kv_in[:].opt()],
            outs=[kv_full[:].opt()],
        )

        # ---------------- q projection (fp32) + transposes ---------------
        def q_consumer(ps, t, d):
            qcb = cbp.tile([P, 512], f32, tag="qcb", name="qcb_sb")
            nc.vector.tensor_copy(qcb, ps)
            for j in range(4):
                h = d * 4 + j
                pst = ps128("pst_q")
                nc.tensor.transpose(pst, qcb[:, j * P:(j + 1) * P], ident)
                nc.vector.tensor_copy(qT[:, h, t * P:(t + 1) * P], pst)

        proj(io["wqT"], F2, FCH, xT_lhsT, q_consumer, False, bias_row=bias_q)

        # ---------------- Phase B: attention per head --------------------
        oT = sheets.tile([P, H, TOK], f32r, tag="sheet", name="oT_sb")
        for h in range(H):
            knat = work.tile([P, KC, P], f32, tag="knat", name="knat_sb")
            nc.sync.dma_start(
                out=knat,
                in_=kv_full[:, h * P:(h + 1) * P]
                .rearrange("(c p) d -> p c d", p=P))
            vh = work.tile([P, KC, P], f32r, tag="vh", name="vh_sb")
            nc.sync.dma_start(
                out=vh,
                in_=kv_full[:, F2 + h * P:F2 + (h + 1) * P].bitcast(f32r)
                .rearrange("(c p) d -> p c d", p=P))
            kT = work.tile([P, KC, P], f32, tag="kT", name="kT_sb")
            for c in range(KC):
                pst = ps128("pst_k")
                nc.tensor.transpose(pst, knat[:, c, :], ident)
                nc.vector.tensor_copy(kT[:, c, :], pst)

            # qtw = [-qi_h; qr_h] for the imaginary part of Q^dag K
            qtw = work.tile([P, TOK], f32, tag="qtw", name="qtw_sb")
            nc.vector.tensor_scalar_mul(qtw[0:HD, :], qT[HD:P, h, :], -1.0)
            nc.vector.tensor_copy(qtw[HD:P, :], qT[0:HD, h, :])

            biast = work.tile([P, TT, S], bf16, tag="biast", name="biast_sb")
            nc.sync.dma_start(
                out=biast,
                in_=io["biasc"][h].rearrange("(t p) k -> p t k", p=P))

            s_sb = work.tile([P, TT, S], f32, tag="s_sb", name="s_sb")
            p_sb = work.tile([P, TT, S], f32, tag="p_sb", name="p_sb")
            nmax = small.tile([P, TT], f32, tag="nmax", name="nmax_sb")
            rsum = small.tile([P, TT], f32, tag="rsum", name="rsum_sb")
            rinv = small.tile([P, TT], f32, tag="rinv", name="rinv_sb")
            for t in range(TT):
                for kc in range(2):
                    ps_r = ps512("ps_sr")
                    ps_i = ps512("ps_si")
                    nc.tensor.matmul(
                        ps_r, qT[:, h, t * P:(t + 1) * P],
                        kT[:, kc * 4:(kc + 1) * 4, :], start=True, stop=True)
                    nc.tensor.matmul(
                        ps_i, qtw[:, t * P:(t + 1) * P],
                        kT[:, kc * 4:(kc + 1) * 4, :], start=True, stop=True)
                    sl = s_sb[:, t, kc * 512:(kc + 1) * 512]
                    nc.scalar.square(sl, ps_r)
                    nc.scalar.square(ps_i, ps_i)
                    nc.vector.tensor_add(sl, sl, ps_i)
                nc.vector.tensor_tensor(
                    s_sb[:, t, :], s_sb[:, t, :], biast[:, t, :],
                    op=mybir.AluOpType.add)
                nc.vector.reduce_max(
                    out=nmax[:, t:t + 1], in_=s_sb[:, t, :],
                    axis=mybir.AxisListType.X, negate=True)
                nc.scalar.activation(
                    out=p_sb[:, t, :], in_=s_sb[:, t, :],
                    func=mybir.ActivationFunctionType.Exp,
                    bias=nmax[:, t:t + 1], scale=1.0,
                    accum_out=rsum[:, t:t + 1])
                nc.vector.reciprocal(rinv[:, t:t + 1], rsum[:, t:t + 1])
                nc.vector.tensor_scalar_mul(
                    p_sb[:, t, :], p_sb[:, t, :], rinv[:, t:t + 1])

            pT = work.tile([P, KC, TOK], f32r, tag="pT", name="pT_sb")
            for t in range(TT):
                for c in range(KC):
                    pst = ps128("pst_p")
                    nc.tensor.transpose(pst, p_sb[:, t, c * P:(c + 1) * P],
                                        ident)
                    nc.vector.tensor_copy(pT[:, c, t * P:(t + 1) * P], pst)

            ps_o = psum.tile([P, TOK], f32, tag="av", space="PSUM", bufs=2,
                             name="ps_av")
            for c in range(KC):
                nc.tensor.matmul(
                    ps_o, vh[:, c, :], pT[:, c, :],
                    start=(c == 0), stop=(c == KC - 1))
            nc.vector.tensor_copy(oT[:, h, :], ps_o)

        # ---------------- Phase C: o_proj + LN1 + x2T --------------------
        x2pre = sheets.tile([P, TT, F2], f32, tag="sheet", name="x2pre_sb")

        def o_consumer(ps, t, d):
            nc.vector.tensor_add(
                x2pre[:, t, d * 512:(d + 1) * 512], ps,
                xnat[:, t, d * 512:(d + 1) * 512])

        def oT_lhsT(f, t):
            return oT[:, f, t * P:(t + 1) * P]

        proj(io["woT"], F2, H, oT_lhsT, o_consumer, True, bias_row=bias_o)

        x2 = sheets.tile([P, TT, F2], f32, tag="sheet", name="x2_sb")
        _layernorm(nc, small, x2, x2pre, eps_t, lnv_sb, 0, flags, bcast)

        x2T = sheets.tile([P, FCH, TOK], f32r, tag="sheet", name="x2T_sb")
        for t in range(TT):
            for f in range(FCH):
                pst = ps128("pst_x2")
                nc.tensor.transpose(pst, x2[:, t, f * P:(f + 1) * P], ident)
                nc.vector.tensor_copy(x2T[:, f, t * P:(t + 1) * P], pst)

        # ---------------- Phase D/E: fused FFN ---------------------------
        y_acc = sheets.tile([P, TT, F2], f32, tag="sheet", name="yacc_sb")

        def x2T_lhsT(f, t):
            return x2T[:, f, t * P:(t + 1) * P]

        for hq in range(4):   # hidden quarters of 2048
            hTq = sheets.tile([P, FCH, TOK], f32r, tag="sheet", name="hTq_sb")

            def h_consumer(ps, t, d, hTq=hTq, hq=hq):
                if flags["any_bias"]:
                    gd = hq * 4 + d
                    nc.vector.tensor_tensor(
                        ps, ps,
                        bcast(b1vec_sb[:, gd * 512:(gd + 1) * 512]),
                        op=mybir.AluOpType.add)
                hcb = cbp.tile([P, 512], f32, tag="hcb", name="hcb_sb")
                nc.scalar.activation(
                    out=hcb, in_=ps, func=mybir.ActivationFunctionType.Relu)
                for j in range(4):
                    fh = d * 4 + j
                    pst = ps128("pst_h")
                    nc.tensor.transpose(pst, hcb[:, j * P:(j + 1) * P], ident)
                    nc.vector.tensor_copy(hTq[:, fh, t * P:(t + 1) * P], pst)

            proj(io["w1T"][:, hq * F2:(hq + 1) * F2], F2, FCH, x2T_lhsT,
                 h_consumer, True)

            def hT_lhsT(f, t, hTq=hTq):
                return hTq[:, f, t * P:(t + 1) * P]

            def y_consumer(ps, t, d, hq=hq):
                if hq == 0:
                    nc.vector.tensor_copy(
                        y_acc[:, t, d * 512:(d + 1) * 512], ps)
                else:
                    nc.vector.tensor_add(
                        y_acc[:, t, d * 512:(d + 1) * 512],
                        y_acc[:, t, d * 512:(d + 1) * 512], ps)

            proj(io["w2T"][hq * F2:(hq + 1) * F2, :], F2, FCH, hT_lhsT,
                 y_consumer, True,
                 bias_row=(bias_f2 if hq == 3 else None))

        # ---------------- residual + LN2 -> output -----------------------
        yout_sb = sheets.tile([P, TT, F2], f32, tag="sheet", name="yout_sb")
        for t in range(TT):
            nc.vector.tensor_add(y_acc[:, t, :], y_acc[:, t, :], x2[:, t, :])
        _layernorm(nc, small, yout_sb, y_acc, eps_t, lnv_sb, 1, flags, bcast)
        nc.sync.dma_start(
            out=io["yout"].rearrange("(t p) f -> p t f", p=P), in_=yout_sb)


def _layernorm(nc, small, out_sb, in_sb, eps_t, lnv_sb, which, flags, bcast):
    """LayerNorm over the last dim (F2) of [P, TT, F2] tiles."""
    NSUB = F2 // 512
    for t in range(TT):
        stats = small.tile([P, NSUB, nc.vector.BN_STATS_DIM], f32,
                           tag="lnstats", name="lnstats_sb")
        for j in range(NSUB):
            nc.vector.bn_stats(out=stats[:, j, :],
                               in_=in_sb[:, t, j * 512:(j + 1) * 512])
        mv = small.tile([P, nc.vector.BN_AGGR_DIM], f32, tag="lnmv",
                        name="lnmv_sb")
        nc.vector.bn_aggr(out=mv, in_=stats)
        rstd = small.tile([P, 1], f32, tag="lnrstd", name="lnrstd_sb")
        nc.scalar.activation(out=rstd, in_=mv[:, 1:2],
                             func=mybir.ActivationFunctionType.Sqrt,
                             bias=eps_t, scale=1.0)
        nc.vector.reciprocal(rstd, rstd)
        nc.vector.tensor_scalar(
            out=out_sb[:, t, :], in0=in_sb[:, t, :],
            scalar1=mv[:, 0:1], scalar2=rstd,
            op0=mybir.AluOpType.subtract, op1=mybir.AluOpType.mult)
        if flags["ln_affine"]:
            g = lnv_sb[:, 2 * which + 0, :]
            b = lnv_sb[:, 2 * which + 1, :]
            nc.vector.tensor_tensor(out_sb[:, t, :], out_sb[:, t, :],
                                    bcast(g), op=mybir.AluOpType.mult)
            nc.vector.tensor_tensor(out_sb[:, t, :], out_sb[:, t, :],
                                    bcast(b), op=mybir.AluOpType.add)


def _host_prep(x, graph_mask, params):
    perm = _head_perm()
    wqT = np.ascontiguousarray(_effT(params["q"], ALPHA)[:, perm])
    wkT = np.ascontiguousarray(_effT(params["k"], ALPHA)[:, perm])
    wvT = np.ascontiguousarray(_effT(params["v"])[:, perm])
    woT = np.ascontiguousarray(_effT(params["o"])[perm, :])
    w1T = _effT(params["ffn1"])
    w2T = _effT(params["ffn2"])

    bq = _bias_eff(params["q"], ALPHA)[perm]
    bk = _bias_eff(params["k"], ALPHA)[perm]
    bv = _bias_eff(params["v"])[perm]
    bo = _bias_eff(params["o"])
    b1 = _bias_eff(params["ffn1"])
    b2 = _bias_eff(params["ffn2"])
    any_bias = any(np.any(b != 0) for b in (bq, bk, bv, bo, b1, b2))

    g1 = np.asarray(params["ln1_g"], np.float32)
    b1n = np.asarray(params["ln1_b"], np.float32)
    g2 = np.asarray(params["ln2_g"], np.float32)
    b2n = np.asarray(params["ln2_b"], np.float32)
    ln_affine = not (np.all(g1 == 1) and np.all(b1n == 0)
                     and np.all(g2 == 1) and np.all(b2n == 0))

    bw = np.asarray(params["bias_weights"], np.float32)
    pos = np.arange(S)
    local = (np.abs(pos[None, :] - pos[:, None]) <= 16).astype(np.float32)
    hier = (np.asarray(graph_mask) > 1).astype(np.float32)  # [B, S, S]

    x = np.asarray(x, np.float32)
    in_maps = []
    for c in range(N_CORES):
        b = c // GROUP
        sl = slice((c % GROUP) * TOK, (c % GROUP + 1) * TOK)
        xs = np.ascontiguousarray(x[b, sl])                      # [TOK, F2]
        xTs = np.ascontiguousarray(xs.T)                         # [F2, TOK]
        biasc = (bw[:, 0][:, None, None] * local[sl][None]
                 + bw[:, 1][:, None, None] * hier[b, sl][None]
                 ).astype(ml_dtypes.bfloat16)
        m = {"xT": xTs, "xnat": xs, "biasc": biasc,
             "wqT": wqT, "wkT": wkT, "wvT": wvT, "woT": woT,
             "w1T": w1T, "w2T": w2T}
        if any_bias:
            m["bvec"] = np.ascontiguousarray(
                np.stack([bq, bk, bv, bo, b2]).astype(np.float32))
            m["b1vec"] = np.ascontiguousarray(b1[None])
        if ln_affine:
            m["lnv"] = np.ascontiguousarray(
                np.stack([g1, b1n, g2, b2n]).astype(np.float32))
        in_maps.append(m)
    flags = {"any_bias": any_bias, "ln_affine": ln_affine}
    return in_maps, flags


def kernel(x, graph_mask, params):
    import os
    global _last_result

    in_maps, flags = _host_prep(x, graph_mask, params)
    key = ("prog", flags["any_bias"], flags["ln_affine"])
    if key not in _cache:
        _cache[key] = _build_program(flags)
    nc = _cache[key]

    trace = bool(os.environ.get("KBENCH_TRACE"))
    res = run_bass_kernel_spmd(nc, in_maps, core_ids=list(range(N_CORES)),
                               trace=trace)
    _last_result = res

    out = np.empty((B, S, F2), np.float32)
    for c in range(N_CORES):
        b = c // GROUP
        sl = slice((c % GROUP) * TOK, (c % GROUP + 1) * TOK)
        out[b, sl] = res.results[c]["yout"]
    return out
